# revision 1
# baseline (speedup 1.0000x reference)
"""Trainium2 Bass kernel for nn_LinearSoftmaxAttention (second-order linear attention).

Math (per batch n, head h; L == S, D == M):
    Q = LN(queries)                       [L,D]
    K = LN(keys) / (3*sqrt(D)) * klen     [S,D]
    KV    = K^T V                         [D,M]
    Ksum  = sum_s K                       [D]
    KK    = K^T K                         [D,D]
    Qsum2 = Q^T Q                         [D,D]
    order1 = Q @ KV                       [L,M]
    norm1  = Q @ Ksum                     [L]
    norm2  = rowsum((Q @ KK) * Q)         [L]
    c      = rowsum((K @ Qsum2) * K)      [S]   (reference contracts QQ over l!)
    order2 = 0.5 * c[:,None] * V          [S,M]
    out = (order1 + order2) / (norm1 + 0.5*norm2)[:,None]

Sharding: one (n,h) pair per NeuronCore -> 8 heads over 8 cores, no collectives.
All per-core inputs are packed host-side into ONE contiguous [128, 388] array so
the kernel needs a single simple DMA (one logical DMA semaphore for consumers).
"""

from contextlib import ExitStack

import numpy as np

import concourse.bacc as bacc
import concourse.mybir as mybir
from concourse import tile
from concourse.bass_utils import run_bass_kernel_spmd
from concourse.masks import make_identity

# Problem constants (hardcoded per harness contract).
L = 512  # query length == key length
D = 32   # head dim == value dim
H = 8    # heads
P = 128  # SBUF partitions
T = L // P  # 4 row-chunks of 128
ALPHA = 3.0
LN_EPS = 1e-5
# K scale folded into the rsqrt: c = 1/(ALPHA*sqrt(D));  c/sqrt(v+eps) =
# 1/sqrt((v+eps)/c^2)  ->  Sqrt(scale=1/c^2, bias=eps/c^2) then reciprocal
_INV_C2 = ALPHA * ALPHA * D  # 1/c^2 = 288

# packed input layout (free-dim columns of the [P, NCOL] input): q|k|klen|v
_QOFF, _KOFF, _LOFF, _VOFF = 0, T * D, 2 * T * D, 2 * T * D + T
NCOL = 3 * T * D + T  # 388

_SUB = mybir.AluOpType.subtract
_MUL = mybir.AluOpType.mult
_ADD = mybir.AluOpType.add
_BYP = mybir.AluOpType.bypass


def _emit(ctx: ExitStack, tc: tile.TileContext, in_d, out_d):
    nc = tc.nc
    f32 = mybir.dt.float32
    X = mybir.AxisListType.X

    consts = ctx.enter_context(tc.tile_pool(name="consts", bufs=1))
    sbuf = ctx.enter_context(tc.tile_pool(name="sbuf", bufs=1))
    psum = ctx.enter_context(tc.tile_pool(name="psum", bufs=1, space="PSUM"))
    psum_tr = ctx.enter_context(tc.tile_pool(name="psum_tr", bufs=2, space="PSUM"))

    identity = consts.tile([P, P], f32)
    make_identity(nc, identity[:])
    eps_q = consts.tile([P, 1], f32)
    eps_k = consts.tile([P, 1], f32)
    dummy = consts.tile([P, 1], f32)
    # Dependency-free Sqrt issued first so Bacc's hoisted act-table load
    # overlaps the input DMA instead of sitting on the LN critical path.
    nc.scalar.activation(dummy[:], nc.const_aps.tensor(0.0, (P, 1)),
                         mybir.ActivationFunctionType.Sqrt)
    nc.gpsimd.memset(eps_q[:], LN_EPS)
    nc.gpsimd.memset(eps_k[:], _INV_C2 * LN_EPS)

    # ---- inputs [P, 388] = [q | k | klen | v]; two DMAs so LN (q,k,klen)
    # starts before v lands (v is only needed later, by phase A) ----
    in_all = sbuf.tile([P, NCOL], f32)
    nc.sync.dma_start(in_all[:, 0:_VOFF], in_d[:, 0:_VOFF])
    nc.scalar.dma_start(in_all[:, _VOFF:NCOL], in_d[:, _VOFF:NCOL])
    q_sb = in_all[:, _QOFF : _QOFF + T * D].rearrange("p (t d) -> p t d", d=D)
    k_sb = in_all[:, _KOFF : _KOFF + T * D].rearrange("p (t d) -> p t d", d=D)
    v_sb = in_all[:, _VOFF : _VOFF + T * D].rearrange("p (t d) -> p t d", d=D)
    klen_sb = in_all[:, _LOFF : _LOFF + T][:, :, None]  # [P, T, 1]

    # work holds [ r2 | Kn | r2 | Qn ] with r2 = sqrt(2) columns:
    # [r2|Kn] is the phase-A2 rhs (-> [r2*Ksum | KK]); [r2|Qn] feeds the nrm
    # reduction, whose 0.5 factor turns (r2*norm1)*r2 back into norm1 while
    # halving the u*Q term. This makes sb_A a single unscaled copy.
    R2 = float(np.sqrt(2.0))
    work = sbuf.tile([P, T, 2 * D + 2], f32)
    nc.gpsimd.memset(work[:, :, 0:1], R2)
    nc.gpsimd.memset(work[:, :, D + 1 : D + 2], R2)
    kn = work[:, :, 1 : D + 1]
    qn = work[:, :, D + 2 : 2 * D + 2]

    # ---- LayerNorm via bn_stats/bn_aggr + fused (x-mu)*rs apply ----
    def layernorm(src, dst_col, sqrt_scale, sqrt_bias, post=None, tag=""):
        stats = sbuf.tile([P, T, 6], f32, tag=f"ln_st{tag}")
        ag = sbuf.tile([P, T, 2], f32, tag=f"ln_ag{tag}")
        rs = sbuf.tile([P, T, 1], f32, tag=f"ln_rs{tag}")
        for t in range(T):
            nc.vector.bn_stats(stats[:, t, :], src[:, t, :])
            nc.vector.bn_aggr(ag[:, t, :], stats[:, t, :])
        # std = sqrt(var*scale + bias) on ACT; then rs = 1/std on DVE
        nc.scalar.activation(rs[:], ag[:, :, 1:2],
                             mybir.ActivationFunctionType.Sqrt,
                             scale=sqrt_scale, bias=sqrt_bias)
        nc.vector.reciprocal(rs[:], rs[:])
        if post is not None:
            nc.vector.tensor_mul(rs[:], rs[:], post)
        for t in range(T):
            nc.vector.tensor_scalar(
                out=work[:, t, dst_col : dst_col + D], in0=src[:, t, :],
                scalar1=ag[:, t, 0:1], scalar2=rs[:, t, 0:1],
                op0=_SUB, op1=_MUL)

    # k first: LN(k) -> phase A -> sb_A -> phase C is the longest chain
    layernorm(k_sb, 1, _INV_C2, eps_k[:], post=klen_sb, tag="k")
    layernorm(q_sb, D + 2, 1.0, eps_q[:], tag="q")

    # ---- qT [32, 512] via PE transpose + ACT copy (both idle here);
    # kT via DVE stream-transpose (32x32 blocks, SBUF->SBUF) ----
    qT = sbuf.tile([D, L], f32)
    kT = sbuf.tile([D, L], f32)
    for t in range(T):
        ptr = psum_tr.tile([D, P], f32, tag="ptr")
        nc.tensor.transpose(ptr[:], qn[:, t, :], identity[:])
        nc.scalar.copy(qT[:, t * P : (t + 1) * P], ptr[:])
    for t in range(T):
        for b in range(P // D):
            rows = slice(D * b, D * (b + 1))
            cols = slice(t * P + D * b, t * P + D * (b + 1))
            nc.vector.transpose(kT[:, cols], kn[rows, t, :])

    # ---- phase A/B: contract over s (accumulate 4 chunks in PSUM) ----
    # psumA[32, 0:D]   = sum_t Kn_t^T @ V_t         = KV
    # psumA[32, D:CAT] = sum_t Kn_t^T @ [r2 | Kn]_t = [r2*Ksum | KK]
    # psumB [32,32]    = sum_t Qn_t^T @ Qn_t        = Qsum2
    CAT = 2 * D + 1  # 65
    psumA1 = psum.tile([D, D], f32)
    psumA2 = psum.tile([D, D + 1], f32)
    psumB = psum.tile([D, D], f32)
    for t in range(T):
        st, sp = (t == 0), (t == T - 1)
        nc.tensor.matmul(psumA1[:], kn[:, t, :], v_sb[:, t, :],
                         start=st, stop=sp)
        nc.tensor.matmul(psumA2[:], kn[:, t, :], work[:, t, 0 : D + 1],
                         start=st, stop=sp)
        nc.tensor.matmul(psumB[:], qn[:, t, :], qn[:, t, :], start=st, stop=sp)

    # sb_A = [KV | r2*Ksum | KK] (unscaled copies), sb_B = 0.5*Qsum2
    sb_A = sbuf.tile([D, CAT], f32)
    sb_B = sbuf.tile([D, D], f32)
    nc.vector.tensor_copy(sb_A[:, 0:D], psumA1[:])
    nc.vector.tensor_copy(sb_A[:, D:CAT], psumA2[:])
    nc.vector.tensor_scalar(out=sb_B[:], in0=psumB[:], scalar1=0.5,
                            scalar2=None, op0=_MUL)

    # ---- phase C/D: contract over d ----
    # psumC[:, t, :] = Q_t @ [KV | Ksum | 0.5KK] = [order1 | norm1 | 0.5u]
    # psumD[:, t, :] = K_t @ (0.5*Qsum2)         = 0.5*t
    psumC = psum.tile([P, T, CAT], f32)
    psumD = psum.tile([P, T, D], f32)
    for t in range(T):
        nc.tensor.matmul(psumC[:, t, :], qT[:, t * P : (t + 1) * P], sb_A[:],
                         start=True, stop=True)
        nc.tensor.matmul(psumD[:, t, :], kT[:, t * P : (t + 1) * P],
                         sb_B[:], start=True, stop=True)

    # ---- epilogue: all ch/nrm reductions first (pipeline behind the
    # per-chunk C/D matmuls), one shared reciprocal, then final pairs ----
    out_sb = sbuf.tile([P, T, D], f32)
    ch = sbuf.tile([P, T, 1], f32)
    nrm = sbuf.tile([P, T, 1], f32)
    for t in range(T):
        s1 = sbuf.tile([P, D], f32, tag="epi_s1", bufs=2)
        s2 = sbuf.tile([P, D + 1], f32, tag="epi_s2", bufs=2)
        # ch_t = rowsum(0.5t * K)
        nc.vector.scalar_tensor_tensor(
            out=s1[:], in0=psumD[:, t, :], scalar=1.0,
            in1=kn[:, t, :], op0=_BYP, op1=_MUL, accum_out=ch[:, t, 0:1])
        # nrm_t = rowsum(0.5*[r2*norm1 | u] * [r2 | Q]) = norm1 + 0.5*norm2
        nc.vector.scalar_tensor_tensor(
            out=s2[:], in0=psumC[:, t, D:CAT], scalar=0.5,
            in1=work[:, t, D + 1 : 2 * D + 2], op0=_MUL, op1=_MUL,
            accum_out=nrm[:, t, 0:1])
    nc.vector.reciprocal(nrm[:], nrm[:])
    for t in range(T):
        s3 = sbuf.tile([P, D], f32, tag="epi_s3", bufs=2)
        # out_t = (V_t*ch_t + order1_t) * rnorm_t
        nc.vector.scalar_tensor_tensor(
            out=s3[:], in0=v_sb[:, t, :], scalar=ch[:, t, 0:1],
            in1=psumC[:, t, 0:D], op0=_MUL, op1=_ADD)
        nc.vector.tensor_scalar(out=out_sb[:, t, :], in0=s3[:],
                                scalar1=nrm[:, t, 0:1], scalar2=None, op0=_MUL)
    nc.sync.dma_start(out_d[:], out_sb[:].rearrange("p t d -> p (t d)"))


_CACHED = {}


def _build():
    if "nc" in _CACHED:
        return _CACHED["nc"]
    # Route every ACT func we use (Sqrt/Copy/Identity/Square) to the single
    # act-func-set that contains them all, so Bacc inserts ONE table load
    # instead of one per first-match set. Set ids are dict positions, which
    # this filter preserves.
    import concourse.hw_specs as hw_specs
    orig_tables = hw_specs.get_activation_tables

    def _tables_one_set(module_arch):
        tabs = orig_tables(module_arch)
        keep = None
        for name, funcs in tabs.items():
            names = {str(f) for f in funcs}
            if any("Sqrt" in s and "Rsqrt" not in s for s in names):
                keep = name
                break
        if keep is None:
            return tabs
        shared = {
            mybir.ActivationFunctionType.Copy,
            mybir.ActivationFunctionType.Identity,
            mybir.ActivationFunctionType.Square,
        }
        return {
            name: (funcs if name == keep else funcs - shared)
            for name, funcs in tabs.items()
        }

    bacc.get_activation_tables = _tables_one_set
    try:
        nc = bacc.Bacc("TRN2", target_bir_lowering=False, debug=False,
                       num_devices=H)
        f32 = mybir.dt.float32
        in_d = nc.dram_tensor("inp", [P, NCOL], f32, kind="ExternalInput")
        out_d = nc.dram_tensor("out", [P, T * D], f32, kind="ExternalOutput")
        with tile.TileContext(nc) as tc:
            with ExitStack() as ctx:
                _emit(ctx, tc, in_d[:], out_d[:])
        nc.compile()
    finally:
        bacc.get_activation_tables = orig_tables
    _CACHED["nc"] = nc
    return nc


def _pack(q, k, v, klen, h):
    # [512, 32] -> [128, 4*32] with col t*32+d = row t*128+p
    def rows(x):
        return np.ascontiguousarray(
            x.reshape(T, P, D).transpose(1, 0, 2).reshape(P, T * D))
    kl = np.ascontiguousarray(klen.reshape(T, P).T)  # [128, 4]
    return np.concatenate(
        [rows(q[0, :, h, :]), rows(k[0, :, h, :]), kl, rows(v[0, :, h, :])],
        axis=1).astype(np.float32)


def kernel(queries, keys, values, attn_mask, query_lengths, key_lengths,
           _want_profile=False, **_ignored):
    nc = _build()
    q = np.asarray(queries, dtype=np.float32)
    k = np.asarray(keys, dtype=np.float32)
    v = np.asarray(values, dtype=np.float32)
    klen = np.asarray(key_lengths, dtype=np.float32)

    in_maps = [{"inp": _pack(q, k, v, klen, h)} for h in range(H)]
    res = run_bass_kernel_spmd(nc, in_maps, list(range(H)),
                               trace=_want_profile)
    # [128, 128] -> [512, 32]
    outs = [
        res.results[h]["out"].reshape(P, T, D).transpose(1, 0, 2).reshape(L, D)
        for h in range(H)
    ]
    out = np.stack(outs, axis=1)[None]
    if _want_profile:
        return out.astype(np.float32), res
    return out.astype(np.float32)



# revision 14
# speedup vs baseline: 1.2333x; 1.2333x over previous
"""Trainium2 Bass kernel for nn_LinearSoftmaxAttention (second-order linear attention).

Math (per batch n, head h; L == S, D == M):
    Q = LN(queries)                       [L,D]
    K = LN(keys) / (3*sqrt(D)) * klen     [S,D]
    psumA  = Kn^T [V | 1 | 1 | Kn]        [D, 2D+2] = [KV | Ksum | Ksum | KK]
    Qsum2  = Qn^T Qn                      [D,D]
    psumC  = Q @ sbA                      [L, 2D+2] = [order1 | norm1 | . | u]
    psumD  = K @ (0.5*Qsum2)              [S,D]
    ch     = rowsum(psumD * K)            [S]     (= 0.5 * c)
    nrm    = norm1 + 0.5*rowsum(u * Q)    [L]
    out    = (order1 + ch*V) / nrm[:,None]

Sharding: one (n,h) pair per NeuronCore -> 8 heads over 8 cores, no collectives.
All matmuls in f16 (inputs are converted host-side; rel-err budget is 2e-2).
Transposes of [Kn|Qn] are done per 128-row chunk on the PE ([128,64] -> [64,128]),
giving kT on partitions 0:32 and qT on partitions 32:64 so that the d-contraction
matmuls (C and D) run as row-group-tiled matmuls without any cross-partition moves.
"""

from contextlib import ExitStack

import numpy as np
import ml_dtypes

import concourse.bacc as bacc
import concourse.mybir as mybir
from concourse import tile
from concourse.bass_utils import run_bass_kernel_spmd
from concourse.masks import make_identity

# Problem constants (hardcoded per harness contract).
L = 512  # query length == key length
D = 32   # head dim == value dim
H = 8    # heads
P = 128  # SBUF partitions
T = L // P  # 4 row-chunks of 128
ALPHA = 3.0
LN_EPS = 1e-5
_INV_C2 = float(ALPHA * ALPHA * D)  # 1/c^2 = 288  (K scale folded into sqrt arg)

# dram input layout (f16 cols): k | q | klen | v
_KOFF, _QOFF, _LOFF, _VOFF = 0, T * D, 2 * T * D, 2 * T * D + T
NCOL = 3 * T * D + T  # 388

# work tile free-dim layout per chunk (f16, 4B-aligned slices):
#   [ V(0:32) | one(32) one(33) | Kn(34:66) | Qn(66:98) | pad ]
_WV, _W1, _WK, _WQ, _WW = 0, D, D + 2, 2 * D + 2, 3 * D + 4  # 0,32,34,66,100
CAT = 2 * D + 2  # 66: A-matmul rhs/psum width [KV | Ksum Ksum | KK]

_SUB = mybir.AluOpType.subtract
_MUL = mybir.AluOpType.mult
_ADD = mybir.AluOpType.add
_BYP = mybir.AluOpType.bypass
_AX = mybir.AxisListType.X


def _emit(ctx: ExitStack, tc: tile.TileContext, in_d, inv_d, out_d):
    nc = tc.nc
    f32 = mybir.dt.float32
    f16 = mybir.dt.float16
    ACT = mybir.ActivationFunctionType

    consts = ctx.enter_context(tc.tile_pool(name="consts", bufs=1))
    sbuf = ctx.enter_context(tc.tile_pool(name="sbuf", bufs=1))
    psum = ctx.enter_context(tc.tile_pool(name="psum", bufs=1, space="PSUM"))
    psum_tr = ctx.enter_context(tc.tile_pool(name="psum_tr", bufs=2, space="PSUM"))

    identity = consts.tile([P, P], f16)
    make_identity(nc, identity[:])
    dummy = consts.tile([P, 1], f32)
    eps_k = consts.tile([P, 1], f32)
    eps_q = consts.tile([P, 1], f32)
    # Dependency-free Sqrt issued first so Bacc's hoisted act-table load
    # overlaps the input DMA instead of sitting on the LN critical path.
    nc.scalar.activation(dummy[:], nc.const_aps.tensor(0.0, (P, 1)), ACT.Sqrt)
    nc.gpsimd.memset(eps_k[:], _INV_C2 * LN_EPS)
    nc.gpsimd.memset(eps_q[:], LN_EPS)

    # ---- input DMAs; raw = [k | q | klen] fp32, v (f16) lands in work ----
    raw = sbuf.tile([P, 2 * T * D + T], f32)  # [128, 260]
    work = sbuf.tile([P, T, _WW], f16)
    nc.gpsimd.memset(work[:, :, _W1 : _W1 + 2], 1.0)
    nc.sync.dma_start(raw[:], in_d[:])
    nc.scalar.dma_start(
        work[:, :, _WV : _WV + D],
        inv_d.rearrange("p (t d) -> p t d", d=D),
    )
    k_raw = raw[:, 0 : T * D].rearrange("p (t d) -> p t d", d=D)
    q_raw = raw[:, T * D : 2 * T * D].rearrange("p (t d) -> p t d", d=D)
    klen = raw[:, 2 * T * D : 2 * T * D + T]  # [128, 4] f32
    kq_g = raw[:, 0 : 2 * T * D].rearrange("p (g d) -> p g d", d=D)  # 8 groups

    # ---- LN stats: s = rowsum(x), ss = rowsum(x^2) per 32-col group ----
    # (k groups 0:4, q groups 4:8). var*32 = ss - s^2/32.
    sq = sbuf.tile([P, 2 * T * D], f16)
    nc.scalar.activation(sq[:], raw[:, 0 : 2 * T * D], ACT.Square)
    s_ = sbuf.tile([P, 2 * T], f32)
    ss = sbuf.tile([P, 2 * T], f32)
    v32 = sbuf.tile([P, 2 * T], f32)
    rs = sbuf.tile([P, 2 * T], f32)   # 1/std (k part includes klen/alpha factors)
    nmu = sbuf.tile([P, 2 * T], f32)  # -mean * rs
    std = sbuf.tile([P, 2 * T], f32)
    nc.vector.tensor_reduce(s_[:], kq_g, axis=_AX, op=_ADD)
    # t1 = -(s/32)*s, reusing v32 as scratch
    nc.vector.scalar_tensor_tensor(out=v32[:], in0=s_[:], scalar=-1.0 / D,
                                   in1=s_[:], op0=_MUL, op1=_MUL)
    nc.vector.tensor_reduce(ss[:], sq[:].rearrange("p (g d) -> p g d", d=D),
                            axis=_AX, op=_ADD)
    nc.vector.tensor_add(v32[:], v32[:], ss[:])
    # k: std_k = sqrt(v32*9 + 288*eps)  (folds the 1/(alpha*sqrt(D)) scale)
    nc.scalar.activation(std[:, 0:T], v32[:, 0:T], ACT.Sqrt,
                         scale=_INV_C2 / D, bias=eps_k[:])
    # q: std_q = sqrt(v32/32 + eps)
    nc.scalar.activation(std[:, T : 2 * T], v32[:, T : 2 * T], ACT.Sqrt,
                         scale=1.0 / D, bias=eps_q[:])
    nc.vector.reciprocal(rs[:, 0:T], std[:, 0:T])
    nc.vector.tensor_mul(rs[:, 0:T], rs[:, 0:T], klen)
    nc.vector.scalar_tensor_tensor(out=nmu[:, 0:T], in0=s_[:, 0:T],
                                   scalar=-1.0 / D, in1=rs[:, 0:T],
                                   op0=_MUL, op1=_MUL)
    nc.vector.reciprocal(rs[:, T : 2 * T], std[:, T : 2 * T])
    nc.vector.scalar_tensor_tensor(out=nmu[:, T : 2 * T], in0=s_[:, T : 2 * T],
                                   scalar=-1.0 / D, in1=rs[:, T : 2 * T],
                                   op0=_MUL, op1=_MUL)

    # ---- LN applies: Kn on ACT (x*rs + nmu), Qn on GpSimd ----
    for t in range(T):
        nc.scalar.activation(work[:, t, _WK : _WK + D], k_raw[:, t, :],
                             ACT.Identity, scale=rs[:, t : t + 1],
                             bias=nmu[:, t : t + 1])
    for t in range(T):
        nc.gpsimd.tensor_scalar(out=work[:, t, _WQ : _WQ + D], in0=q_raw[:, t, :],
                                scalar1=rs[:, T + t : T + t + 1],
                                scalar2=nmu[:, T + t : T + t + 1],
                                op0=_MUL, op1=_ADD)

    # ---- PE: A (into psum partitions 32:64), B, per-chunk [Kn|Qn] transpose ----
    psumA = psum.tile([2 * D, CAT], f32)      # rows 32:64 used
    psumB = psum.tile([D, D], f32)
    kqT = sbuf.tile([2 * D, T, P], f16)      # rows 0:32 kT, 32:64 qT
    pT = [None] * T
    for t in range(T):
        st, sp = (t == 0), (t == T - 1)
        nc.tensor.matmul(psumA[D : 2 * D, :], work[:, t, _WK : _WK + D],
                         work[:, t, 0:CAT], start=st, stop=sp,
                         tile_position=(0, D))
        nc.tensor.matmul(psumB[:], work[:, t, _WQ : _WQ + D],
                         work[:, t, _WQ : _WQ + D], start=st, stop=sp)
        ptile = psum_tr.tile([2 * D, P], f16, tag="ptr")
        pT[t] = ptile
        nc.tensor.transpose(ptile[:], work[:, t, _WK : _WK + 2 * D], identity[:])

    # evacuations: sbA (f16, partitions 32:64) on DVE; sbB = 0.5*Qsum2 on ACT;
    # kqT chunk copies split ACT/DVE
    sbA = sbuf.tile([2 * D, CAT], f16)
    sbB = sbuf.tile([D, D], f16)
    nc.scalar.activation(sbB[:], psumB[:], ACT.Copy, scale=0.5)
    nc.scalar.copy(kqT[:, 0, :], pT[0][:])
    nc.vector.tensor_copy(kqT[:, 1, :], pT[1][:])
    nc.vector.tensor_copy(sbA[D : 2 * D, :], psumA[D : 2 * D, :])
    nc.scalar.copy(kqT[:, 2, :], pT[2][:])
    nc.vector.tensor_copy(kqT[:, 3, :], pT[3][:])

    # ---- PE: D then C (row-group-tiled over d) ----
    psumD = psum.tile([P, T, D], f32)
    psumC = psum.tile([P, T, CAT], f32)
    for t in range(T):
        nc.tensor.matmul(psumD[:, t, :], kqT[0:D, t, :], sbB[:],
                         start=True, stop=True)
    for t in range(T):
        nc.tensor.matmul(psumC[:, t, :], kqT[D : 2 * D, t, :], sbA[D : 2 * D, :],
                         start=True, stop=True, tile_position=(D, 0))

    # ---- epilogue ----
    # ch = rowsum(psumD * Kn)  (starts while C matmuls still run)
    e1 = sbuf.tile([P, T, D], f32)
    ch = sbuf.tile([P, T], f32)
    nc.vector.tensor_mul(e1[:], psumD[:], work[:, :, _WK : _WK + D])
    nc.vector.tensor_reduce(ch[:], e1[:], axis=_AX, op=_ADD)
    # nrm = norm1 + 0.5*rowsum(u * Qn); rnorm = 1/nrm
    e2 = sbuf.tile([P, T, D], f32)
    nrm = sbuf.tile([P, T], f32)
    nc.vector.tensor_mul(e2[:], psumC[:, :, _WK:CAT], work[:, :, _WQ : _WQ + D])
    nc.vector.tensor_reduce(nrm[:], e2[:], axis=_AX, op=_ADD)
    nc.vector.scalar_tensor_tensor(out=nrm[:], in0=nrm[:], scalar=0.5,
                                   in1=psumC[:, :, D], op0=_MUL, op1=_ADD)
    nc.vector.reciprocal(nrm[:], nrm[:])
    # out_t = (V*ch + order1) * rnorm ; DVE does the STT, ACT the final scale
    out_sb = sbuf.tile([P, T, D], f32)
    for t in range(T):
        s3 = sbuf.tile([P, D], f32, tag="epi_s3", bufs=2)
        nc.vector.scalar_tensor_tensor(
            out=s3[:], in0=work[:, t, _WV : _WV + D], scalar=ch[:, t : t + 1],
            in1=psumC[:, t, 0:D], op0=_MUL, op1=_ADD)
        nc.scalar.activation(out_sb[:, t, :], s3[:], ACT.Identity,
                             scale=nrm[:, t : t + 1])
        dma_eng = (nc.sync, nc.gpsimd, nc.sync, nc.gpsimd)[t]
        dma_eng.dma_start(out_d[:, t * D : (t + 1) * D], out_sb[:, t, :])


_CACHED = {}


def _build():
    if "nc" in _CACHED:
        return _CACHED["nc"]
    # Route every ACT func we use (Sqrt/Copy/Identity/Square) to the single
    # act-func-set that contains them all, so Bacc inserts ONE table load
    # instead of one per first-match set.
    import concourse.hw_specs as hw_specs
    orig_tables = hw_specs.get_activation_tables

    def _tables_one_set(module_arch):
        tabs = orig_tables(module_arch)
        keep = None
        for name, funcs in tabs.items():
            names = {str(f) for f in funcs}
            if any("Sqrt" in s and "Rsqrt" not in s for s in names):
                keep = name
                break
        if keep is None:
            return tabs
        shared = {
            mybir.ActivationFunctionType.Copy,
            mybir.ActivationFunctionType.Identity,
            mybir.ActivationFunctionType.Square,
        }
        return {
            name: (funcs if name == keep else funcs - shared)
            for name, funcs in tabs.items()
        }

    bacc.get_activation_tables = _tables_one_set
    try:
        nc = bacc.Bacc("TRN2", target_bir_lowering=False, debug=False,
                       num_devices=H)
        f32 = mybir.dt.float32
        f16 = mybir.dt.float16
        in_d = nc.dram_tensor("inp", [P, 2 * T * D + T], f32,
                              kind="ExternalInput")
        inv_d = nc.dram_tensor("inpv", [P, T * D], f16, kind="ExternalInput")
        out_d = nc.dram_tensor("out", [P, T * D], f32, kind="ExternalOutput")
        with tile.TileContext(nc) as tc:
            with ExitStack() as ctx:
                _emit(ctx, tc, in_d[:], inv_d[:], out_d[:])
        nc.compile()
    finally:
        bacc.get_activation_tables = orig_tables
    _CACHED["nc"] = nc
    return nc


def _pack(q, k, v, klen, h):
    # [512, 32] -> [128, 4*32] with col t*32+d = row t*128+p
    def rows(x):
        return np.ascontiguousarray(
            x.reshape(T, P, D).transpose(1, 0, 2).reshape(P, T * D))
    kl = np.ascontiguousarray(klen.reshape(T, P).T)  # [128, 4]
    kq = np.concatenate(
        [rows(k[0, :, h, :]), rows(q[0, :, h, :]), kl], axis=1)
    return (kq.astype(np.float32), rows(v[0, :, h, :]).astype(np.float16))


def kernel(queries, keys, values, attn_mask, query_lengths, key_lengths,
           _want_profile=False, **_ignored):
    nc = _build()
    q = np.asarray(queries, dtype=np.float32)
    k = np.asarray(keys, dtype=np.float32)
    v = np.asarray(values, dtype=np.float32)
    klen = np.asarray(key_lengths, dtype=np.float32)

    packed = [_pack(q, k, v, klen, h) for h in range(H)]
    in_maps = [{"inp": kq, "inpv": vv} for kq, vv in packed]
    res = run_bass_kernel_spmd(nc, in_maps, list(range(H)),
                               trace=_want_profile)
    # [128, 128] -> [512, 32]
    outs = [
        np.asarray(res.results[h]["out"], dtype=np.float32)
        .reshape(P, T, D).transpose(1, 0, 2).reshape(L, D)
        for h in range(H)
    ]
    out = np.stack(outs, axis=1)[None]
    if _want_profile:
        return out.astype(np.float32), res
    return out.astype(np.float32)
